# revision 1
# baseline (speedup 1.0000x reference)
"""Trainium2 Bass kernel for llama-style GQA attention layer (B=2, S=1024,
D=4096, H=32, KVH=8, HD=128, start_pos=0), tensor-parallel over heads on 8
NeuronCores.

Per-core plan (core c of 8):
  - owns q heads 4c..4c+3 (wq cols 512c..512c+512) and kv head c
    (wk/wv cols 128c..128c+128); GQA grouping means exactly its heads use
    exactly its kv head.
  - computes qT/kT/vT = (x @ w).T directly in [col, token] layout via
    w-stationary matmuls against host-transposed xT.
  - RoPE applied in qT/kT layout; the head-dim interleave is de-interleaved
    by permuting wq/wk columns on the host (a consistent permutation of the
    hd axis of q AND k leaves q.k dot products unchanged).
  - attention computed in transposed orientation: ST[k, q] = kT_chunk.T-free
    matmuls, softmax without max-subtraction (scores are O(10) here so fp32
    exp is safe), row sums via ones-vector matmul, PV with v-stationary
    matmuls giving outT [hd, q]; normalization by broadcasting 1/rowsum via
    a K=1 outer-product matmul.
  - heads' outT concat -> AllGather (per batch) -> full attn_outT on every
    core -> each core computes a 512-column slice of the wo projection
    (no all-reduce needed: the head contraction happens inside the matmul).
Host unshards by concatenating the 8 column slices and transposing.

Projection and wo matmuls run in bf16 (PE-native: overlapped weight loads,
half the DMA bytes) with fp32 PSUM accumulation; the attention core
(scores, exp, PV, normalization) stays fp32r/fp32.

Measured on the 8-core axon trn2 pod (R-replication timing, same-session
A/B): ~580 us device time per invocation (fp32r everywhere: ~670 us),
relative error vs the fp32 jax reference 3.6e-3 (fp32r: 2.7e-4 — flip
USE_BF16 to trade speed for precision).
"""

import numpy as np
import concourse.mybir as mybir
import concourse.tile as tile
from concourse import bacc
from concourse.bass_utils import run_bass_kernel_spmd

N_CORES = 8
B, S, D = 2, 1024, 4096
TOK = B * S            # 2048 flattened tokens
HD = 128
NHC = 4                # q heads per core
COLS = 6               # projection col chunks per core: 4 q heads + k + v
F32 = mybir.dt.float32
F32R = mybir.dt.float32r
BF16 = mybir.dt.bfloat16
USE_BF16 = True          # bf16 projection/wo matmuls (fp32 PSUM accumulate)
AG_SPLIT = False         # 2 per-batch AllGathers measured faster than 4 halves
WDT = BF16 if USE_BF16 else F32R
SCALE = 1.0 / float(np.sqrt(128.0))
NEG = -1e9

_CACHE = {}


def _emit(nc, tc, aps, collective=True, phases=3, sfx=""):
    xt, wqkv, wo, cost, sint, mask, iden, perm, ones, onesr, y = aps
    nag = 2 if AG_SPLIT else 1
    agw = 1024 // nag
    ag_in = [
        [
            nc.dram_tensor(f"ag_in{b}_{t}{sfx}", [512, agw], WDT)
            for t in range(nag)
        ]
        for b in range(B)
    ]
    ag_out = [
        [
            nc.dram_tensor(
                f"ag_out{b}_{t}{sfx}", [D, agw], WDT, addr_space="Shared"
            )
            for t in range(nag)
        ]
        for b in range(B)
    ]
    EXP = mybir.ActivationFunctionType.Exp

    with tc.tile_pool(name="persist" + sfx, bufs=1) as pp:
        qkv = [pp.tile([128, TOK], F32R, tag=f"qkv{c}", name=f"qkv{c}") for c in range(COLS)]
        mask_sb = pp.tile([128, 128], F32R, tag="mask")
        iden_sb = pp.tile([128, 128], F32R, tag="iden")
        perm_sb = pp.tile([128, 128], F32R, tag="perm")
        ones_sb = pp.tile([128, 1], F32R, tag="ones")
        onesr_sb = pp.tile([1, 128], F32R, tag="onesr")
        nc.gpsimd.dma_start(out=mask_sb, in_=mask[:, :].bitcast(F32R))
        nc.gpsimd.dma_start(out=iden_sb, in_=iden[:, :].bitcast(F32R))
        nc.gpsimd.dma_start(out=perm_sb, in_=perm[:, :].bitcast(F32R))
        nc.gpsimd.dma_start(out=ones_sb, in_=ones[:, :].bitcast(F32R))
        nc.gpsimd.dma_start(out=onesr_sb, in_=onesr[:, :].bitcast(F32R))

        # ---------------- phase 1: fused QKV projection + RoPE ----------------
        with tc.tile_pool(name="ph1" + sfx, bufs=1) as p1, tc.tile_pool(
            name="ps1", bufs=8, space="PSUM"
        ) as ps1:
            cos_sb = p1.tile([128, TOK], F32R, tag="cos")
            sin_sb = p1.tile([128, TOK], F32R, tag="sin")
            nc.gpsimd.dma_start(out=cos_sb, in_=cost[:, :].bitcast(F32R))
            nc.gpsimd.dma_start(out=sin_sb, in_=sint[:, :].bitcast(F32R))
            wt = [None] * 8
            for t in range(4):
                tsl = slice(t * 512, (t + 1) * 512)
                psl = [ps1.tile([128, 512], F32, tag="proj", name=f"proj{t}_{ci}", bufs=7) for ci in range(COLS)]
                for jg2 in range(16):
                    if t == 0 and jg2 % 2 == 0:
                        jg = jg2 // 2
                        w_ = p1.tile(
                            [128, 4, 768], WDT, tag=f"w{jg}", name=f"w{jg}"
                        )
                        nc.sync.dma_start(
                            out=w_,
                            in_=wqkv[jg * 512 : (jg + 1) * 512, :]
                            .rearrange("(jj p) n -> p jj n", p=128)
                            .bitcast(WDT),
                        )
                        wt[jg] = w_
                    xs = p1.tile([128, 2, 512], WDT, tag="xs", bufs=4)
                    nc.sync.dma_start(
                        out=xs,
                        in_=xt[jg2 * 256 : (jg2 + 1) * 256, tsl]
                        .rearrange("(jj p) s -> p jj s", p=128)
                        .bitcast(WDT),
                    )
                    for jj in range(2):
                        jja = jg2 * 2 + jj - (jg2 // 2) * 4
                        for c in range(COLS):
                            nc.tensor.matmul(
                                psl[c],
                                wt[jg2 // 2][:, jja, c * 128 : (c + 1) * 128],
                                xs[:, jj, :],
                                start=(jg2 == 0 and jj == 0),
                                stop=(jg2 == 15 and jj == 1),
                            )
                for c in range(COLS):
                    nc.scalar.copy(qkv[c][:, tsl], psl[c])
                # RoPE on the 4 q heads + k (qkv[5] = v stays raw).  Rows
                # 0:64 hold the de-interleaved even (real) lanes, 64:128 the
                # odd (imag) lanes.  out = x*[c;c] + swap(x)*[-s;s], with the
                # half-swap done on the PE (DVE cannot cross base partitions).
                for c in range(5):
                    xsw = ps1.tile([128, 512], F32, tag="xsw", bufs=1)
                    nc.tensor.matmul(
                        xsw, perm_sb, qkv[c][:, tsl], start=True, stop=True
                    )
                    m1 = p1.tile([128, 512], F32R, tag="rt", bufs=4)
                    m2 = p1.tile([128, 512], F32R, tag="rt", bufs=4)
                    nc.vector.tensor_mul(m1, qkv[c][:, tsl], cos_sb[:, tsl])
                    nc.vector.tensor_mul(m2, xsw.bitcast(F32R), sin_sb[:, tsl])
                    nc.vector.tensor_add(qkv[c][:, tsl], m1, m2)

        if phases < 2:
            return
        # ---------------- phase 2: attention + AG + wo ----------------
        with tc.tile_pool(name="ph2" + sfx, bufs=1) as p2, tc.tile_pool(
            name="ps2", bufs=1, space="PSUM"
        ) as ps2:
            wt2 = []
            for jg in range(8):
                w_ = p2.tile([128, 4, 512], WDT, tag=f"wo{jg}")
                nc.gpsimd.dma_start(
                    out=w_,
                    in_=wo[jg * 512 : (jg + 1) * 512, :]
                    .rearrange("(jj p) n -> p jj n", p=128)
                    .bitcast(WDT),
                )
                wt2.append(w_)
            # v in natural [token, hd] layout via PE transposes
            vsb = {}
            for b in range(B):
                for j in range(8):
                    trp = ps2.tile([128, 128], F32R, tag="st", bufs=2)
                    nc.tensor.transpose(
                        trp,
                        qkv[5][:, b * 1024 + j * 128 : b * 1024 + (j + 1) * 128],
                        iden_sb,
                    )
                    v_ = p2.tile([128, 128], F32R, tag="vsb", bufs=16)
                    nc.scalar.copy(v_, trp)
                    vsb[(b, j)] = v_
            ao = [p2.tile([128, TOK], WDT, tag=f"ao{h}", name=f"ao{h}") for h in range(NHC)]
            for b in range(B):
                for h in range(NHC):
                    qT = qkv[h]
                    kT = qkv[4]
                    for t2 in range(2):
                        qbase = b * 1024 + t2 * 512
                        jmax = (t2 + 1) * 4
                        ptag = "ov" if (h * 2 + t2) % 2 == 0 else "flex"
                        outp = ps2.tile([128, 512], F32, tag="ov", bufs=2,
                                        name=f"outp{b}_{h}_{t2}")
                        sums = ps2.tile([1, 512], F32, tag="ov", bufs=2,
                                        name=f"sums{b}_{h}_{t2}")
                        for j in range(jmax):
                            qstart = max(t2 * 512, j * 128)
                            width = t2 * 512 + 512 - qstart
                            st_ps = ps2.tile([128, 512], F32, tag="st", bufs=2)
                            nc.tensor.matmul(
                                st_ps[:, :width],
                                kT[:, b * 1024 + j * 128 : b * 1024 + (j + 1) * 128],
                                qT[:, b * 1024 + qstart : b * 1024 + qstart + width],
                                start=True,
                                stop=True,
                            )
                            pexp = p2.tile([128, 512], F32R, tag="pexp", bufs=2)
                            nc.scalar.activation(
                                pexp[:, :width], st_ps[:, :width], EXP, scale=SCALE
                            )
                            if j * 128 >= t2 * 512:
                                # zero the non-causal lower triangle of the
                                # diagonal block (mask_sb is 0/1 here)
                                nc.vector.tensor_mul(
                                    pexp[:, 0:128], pexp[:, 0:128], mask_sb
                                )
                            off = qstart - t2 * 512
                            nc.tensor.matmul(
                                outp[:, off : off + width],
                                vsb[(b, j)],
                                pexp[:, :width],
                                start=(j == 0),
                                stop=(j == jmax - 1),
                            )
                            nc.tensor.matmul(
                                sums[:, off : off + width],
                                ones_sb,
                                pexp[:, :width],
                                start=(j == 0),
                                stop=(j == jmax - 1),
                            )
                        rec = p2.tile([1, 512], F32, tag="rec", bufs=1)
                        nc.vector.reciprocal(rec, sums)
                        recr = p2.tile([1, 512], F32R, tag="recr", bufs=1)
                        nc.scalar.copy(recr, rec)
                        bc = ps2.tile([128, 512], F32, tag="st", bufs=2)
                        nc.tensor.matmul(bc, onesr_sb, recr, start=True, stop=True)
                        bcs = p2.tile([128, 512], F32, tag="bcs", bufs=1)
                        nc.scalar.copy(bcs, bc)
                        nc.vector.tensor_mul(
                            ao[h][:, qbase : qbase + 512], outp, bcs
                        )
                for t2 in range(nag):
                    for h in range(NHC):
                        nc.gpsimd.dma_start(
                            out=ag_in[b][t2][h * 128 : (h + 1) * 128, :],
                            in_=ao[h][
                                :, b * 1024 + t2 * agw : b * 1024 + (t2 + 1) * agw
                            ],
                        )
                    if collective:
                        nc.gpsimd.collective_compute(
                            "AllGather",
                            mybir.AluOpType.bypass,
                            ins=[ag_in[b][t2][:, :]],
                            outs=[ag_out[b][t2][:, :]],
                            replica_groups=[list(range(N_CORES))],
                        )
                    else:
                        nc.gpsimd.dma_start(
                            out=ag_out[b][t2][0:512, :], in_=ag_in[b][t2][:, :]
                        )
            # wo projection: y[dcol, tok] = wo_cols.T @ attn_outT_full
            if phases < 3:
                return
            for b in range(B):
                for t2 in range(2):
                    yp = [ps2.tile([128, 512], F32, tag=f"y{d}", bufs=1, name=f"yp{b}_{t2}_{d}") for d in range(4)]
                    for jg in range(8):
                        ags = p2.tile([128, 4, 512], WDT, tag="ags", bufs=3)
                        agsrc = (
                            ag_out[b][t2]
                            if AG_SPLIT
                            else ag_out[b][0][:, t2 * 512 : (t2 + 1) * 512]
                        )
                        nc.gpsimd.dma_start(
                            out=ags,
                            in_=agsrc[jg * 512 : (jg + 1) * 512, :]
                            .rearrange("(jj p) s -> p jj s", p=128)
                            if AG_SPLIT
                            else ag_out[b][0][
                                jg * 512 : (jg + 1) * 512, t2 * 512 : (t2 + 1) * 512
                            ].rearrange("(jj p) s -> p jj s", p=128),
                        )
                        for jj in range(4):
                            for d in range(4):
                                nc.tensor.matmul(
                                    yp[d],
                                    wt2[jg][:, jj, d * 128 : (d + 1) * 128],
                                    ags[:, jj, :],
                                    start=(jg == 0 and jj == 0),
                                    stop=(jg == 7 and jj == 3),
                                )
                    for d in range(4):
                        ys = p2.tile([128, 512], F32, tag="ys", bufs=2)
                        if d % 2 == 0:
                            nc.scalar.copy(ys, yp[d])
                        else:
                            nc.vector.tensor_copy(ys, yp[d])
                        nc.gpsimd.dma_start(
                            out=y[
                                d * 128 : (d + 1) * 128,
                                b * 1024 + t2 * 512 : b * 1024 + (t2 + 1) * 512,
                            ],
                            in_=ys,
                        )


def _build(single=False, phases=3):
    key = ("nc_single" if single else "nc") + str(phases)
    if key in _CACHE:
        return _CACHE[key]
    nc = bacc.Bacc(
        "TRN2",
        target_bir_lowering=False,
        debug=False,
        num_devices=1 if single else N_CORES,
    )
    xt = nc.declare_dram_parameter("xt", [D, TOK], WDT, isOutput=False)
    wqkv = nc.declare_dram_parameter("wqkv", [D, 768], WDT, isOutput=False)
    wo = nc.declare_dram_parameter("wo", [D, 512], WDT, isOutput=False)
    cost = nc.declare_dram_parameter("cost", [128, TOK], F32, isOutput=False)
    sint = nc.declare_dram_parameter("sint", [128, TOK], F32, isOutput=False)
    mask = nc.declare_dram_parameter("mask", [128, 128], F32, isOutput=False)
    iden = nc.declare_dram_parameter("iden", [128, 128], F32, isOutput=False)
    perm = nc.declare_dram_parameter("perm", [128, 128], F32, isOutput=False)
    ones = nc.declare_dram_parameter("ones", [128, 1], F32, isOutput=False)
    onesr = nc.declare_dram_parameter("onesr", [1, 128], F32, isOutput=False)
    y = nc.declare_dram_parameter("y", [512, TOK], F32, isOutput=True)
    with tile.TileContext(nc) as tc:
        _emit(
            nc,
            tc,
            (xt, wqkv, wo, cost, sint, mask, iden, perm, ones, onesr, y),
            collective=not single,
            phases=phases,
        )
    nc.compile()
    _CACHE[key] = nc
    return nc


def _host_inputs(x, wq, wk, wv, wo, freqs_cos, freqs_sin):
    """Build the per-core input maps (host-side sharding/layout prep)."""
    import ml_dtypes

    wnp = ml_dtypes.bfloat16 if USE_BF16 else np.float32
    xt = np.ascontiguousarray(x.reshape(TOK, D).T).astype(wnp)  # [D, TOK]
    # de-interleave permutation of the head dim for q/k weight columns
    perm = np.concatenate([np.arange(0, HD, 2), np.arange(1, HD, 2)])
    cos_t = np.tile(freqs_cos.T, (1, B))  # [64, TOK]
    sin_t = np.tile(freqs_sin.T, (1, B))
    cost = np.concatenate([cos_t, cos_t], axis=0).astype(np.float32)  # [128, TOK]
    sint = np.concatenate([-sin_t, sin_t], axis=0).astype(np.float32)
    permm = np.zeros((128, 128), np.float32)
    permm[np.arange(64), np.arange(64) + 64] = 1.0
    permm[np.arange(64) + 64, np.arange(64)] = 1.0
    kq, qq = np.meshgrid(np.arange(128), np.arange(128), indexing="ij")
    mask = np.where(qq >= kq, 1.0, 0.0).astype(np.float32)  # [k, q]
    iden = np.eye(128, dtype=np.float32)
    ones = np.ones((128, 1), np.float32)
    onesr = np.ones((1, 128), np.float32)

    in_maps = []
    for c in range(N_CORES):
        wq_c = wq[:, c * 512 : (c + 1) * 512].reshape(D, NHC, HD)[:, :, perm]
        wq_c = wq_c.reshape(D, NHC * HD)
        wk_c = wk[:, c * 128 : (c + 1) * 128][:, perm]
        wv_c = wv[:, c * 128 : (c + 1) * 128]
        wqkv_c = np.ascontiguousarray(
            np.concatenate([wq_c, wk_c, wv_c], axis=1)
        ).astype(wnp)  # [D, 768]
        wo_c = np.ascontiguousarray(wo[:, c * 512 : (c + 1) * 512]).astype(wnp)
        in_maps.append(
            {
                "xt": xt,
                "wqkv": wqkv_c,
                "wo": wo_c,
                "cost": cost,
                "sint": sint,
                "mask": mask,
                "iden": iden,
                "perm": permm,
                "ones": ones,
                "onesr": onesr,
            }
        )
    return in_maps


def kernel(
    x,
    wq,
    wk,
    wv,
    wo,
    freqs_cos,
    freqs_sin,
    cache_k=None,
    cache_v=None,
    start_pos=0,
):
    # start_pos is 0 in this problem; the cache read-back region is then
    # exactly the freshly written k/v, so the caches never matter.
    assert int(start_pos) == 0
    x = np.asarray(x, np.float32)
    in_maps = _host_inputs(
        x,
        np.asarray(wq, np.float32),
        np.asarray(wk, np.float32),
        np.asarray(wv, np.float32),
        np.asarray(wo, np.float32),
        np.asarray(freqs_cos, np.float32),
        np.asarray(freqs_sin, np.float32),
    )
    nc = _build()
    res = run_bass_kernel_spmd(nc, in_maps, list(range(N_CORES))).results
    y_t = np.concatenate([res[c]["y"] for c in range(N_CORES)], axis=0)  # [D, TOK]
    return np.ascontiguousarray(y_t.T).reshape(B, S, D).astype(np.float32)



# revision 16
# speedup vs baseline: 1.8548x; 1.8548x over previous
"""Trainium2 Bass kernel for llama-style GQA attention layer (B=2, S=1024,
D=4096, H=32, KVH=8, HD=128, start_pos=0), tensor-parallel over heads on 8
NeuronCores.

Per-core plan (core c of 8):
  - owns q heads 4c..4c+3 (wq cols 512c..512c+512) and kv head c
    (wk/wv cols 128c..128c+128); GQA grouping means exactly its heads use
    exactly its kv head.
  - computes qT/kT/vT = (x @ w).T directly in [col, token] layout via
    w-stationary matmuls against host-transposed xT.
  - RoPE applied in qT/kT layout; the head-dim interleave is de-interleaved
    by permuting wq/wk columns on the host (a consistent permutation of the
    hd axis of q AND k leaves q.k dot products unchanged).  The half-swap
    runs as an SBUF->SBUF partition-crossing DMA (not on the PE).
  - attention computed in transposed orientation: ST[k, q] from kT-stationary
    matmuls, softmax without max-subtraction (scores are O(10) here so fp32
    exp is safe), PV with v-stationary matmuls giving outT [hd, q]; softmax
    denominators come from an all-ones [128,128]-stationary matmul whose
    output IS the row-sum broadcast across partitions (no separate
    broadcast step), then reciprocal+multiply on the DVE.
  - heads' outT concat -> AllGather (per batch) -> full attn_outT on every
    core -> each core computes a 512-column slice of the wo projection
    (no all-reduce needed: the head contraction happens inside the matmul).
Host unshards by concatenating the 8 column slices and transposing.

The whole kernel is emitted as ONE interleaved stream: for each 512-token
chunk, projection -> RoPE -> that chunk's attention unit, with the wo
projection units at the end.  The Tile scheduler then fills attention's
exp/normalization dependency stalls with projection / wo matmuls, keeping
the PE (the bottleneck engine: ~320 us of matmul work per core) dense
instead of serializing three phases.  PSUM is split 4/2/2 banks:
projection runs its 6 output columns in two groups of 3 (streaming xT from
DRAM twice) sharing a 4-slot accumulator tag with the wo units, scores get
2, and out/sums accumulators 2.

Projection and wo matmuls run in bf16 (fp32 PSUM accumulation); the score
path (q/k, exp input) stays fp32r/fp32, and the post-softmax path (probs,
v) is bf16.
"""

import numpy as np
import concourse.mybir as mybir
import concourse.tile as tile
from concourse import bacc
from concourse.bass_utils import run_bass_kernel_spmd

N_CORES = 8
B, S, D = 2, 1024, 4096
TOK = B * S            # 2048 flattened tokens
HD = 128
NHC = 4                # q heads per core
CH = 4                 # 512-token chunks
F32 = mybir.dt.float32
F32R = mybir.dt.float32r
BF16 = mybir.dt.bfloat16
USE_BF16 = True          # bf16 projection/wo matmuls (fp32 PSUM accumulate)
WDT = BF16 if USE_BF16 else F32R
SCALE = 1.0 / float(np.sqrt(128.0))

_CACHE = {}


def _emit(nc, tc, aps, collective=True, phases=3, sfx=""):
    xt, wqkv, wo, cost, sint, mask, iden, ones, y = aps
    # one AllGather per 512-token chunk: fires as soon as that chunk's
    # attention completes, so every collective has a long compute window
    # (the following chunks' projection/attention + earlier wo units) to
    # hide under.
    ag_in = [nc.dram_tensor(f"ag_in{t}{sfx}", [512, 512], WDT) for t in range(CH)]
    ag_out = [
        nc.dram_tensor(f"ag_out{t}{sfx}", [D, 512], WDT, addr_space="Shared")
        for t in range(CH)
    ]
    EXP = mybir.ActivationFunctionType.Exp

    with tc.tile_pool(name="pp" + sfx, bufs=1) as pp, tc.tile_pool(
        name="ps" + sfx, bufs=1, space="PSUM"
    ) as ps:
        mask_sb = pp.tile([128, 128], BF16, tag="mask")
        iden_sb = pp.tile([128, 128], F32R, tag="iden")
        ones_sb = pp.tile([128, 128], BF16, tag="ones")
        nc.gpsimd.dma_start(out=mask_sb, in_=mask[:, :])
        nc.gpsimd.dma_start(out=iden_sb, in_=iden[:, :].bitcast(F32R))
        nc.gpsimd.dma_start(out=ones_sb, in_=ones[:, :])

        # persistent SBUF state
        qkv = [
            [pp.tile([128, 512], F32R, tag=f"qkv{c}_{t}", name=f"qkv{c}_{t}") for t in range(CH)]
            for c in range(5)
        ]  # 4 q heads + k, per chunk, RoPE'd in place
        vsb = {}  # (b, j) -> [tok128, hd128] bf16 v blocks
        ao = [pp.tile([128, TOK], WDT, tag=f"ao{h}", name=f"ao{h}") for h in range(NHC)]
        wt = [None] * 8
        wt2 = [pp.tile([128, 4, 512], WDT, tag=f"wo{jg}", name=f"wo{jg}") for jg in range(8)]

        def proj_chunk(t):
            tsl = slice(t * 512, (t + 1) * 512)
            cos_sb = pp.tile([128, 512], F32R, tag="cos", bufs=2)
            sin_sb = pp.tile([128, 512], F32R, tag="sin", bufs=2)
            nc.gpsimd.dma_start(out=cos_sb, in_=cost[:, tsl].bitcast(F32R))
            nc.gpsimd.dma_start(out=sin_sb, in_=sint[:, tsl].bitcast(F32R))
            vtmp = pp.tile([128, 512], F32R, tag="vtmp", bufs=2, name=f"vtmp{t}")
            # Chunk 0 runs all 6 output columns in one pass over xT
            # (borrowing 2 idle score banks — attention hasn't started yet)
            # to halve the DMA-bound startup window.  Later chunks use two
            # col-groups of 3 so only 4 accumulator banks are needed while
            # attention/wo own the rest; xT streams once per group.
            groups = [(0, 1, 2, 3, 4, 5)] if t == 0 else [(0, 1, 2), (3, 4, 5)]
            for cols in groups:
                psl = {
                    c: ps.tile(
                        [128, 512], F32,
                        tag=("acc" if c < 4 or t > 0 else "st"),
                        bufs=(4 if c < 4 or t > 0 else 2),
                        name=f"psl{t}_{c}",
                    )
                    for c in cols
                }
                for jg2 in range(16):
                    xs = pp.tile([128, 2, 512], WDT, tag="xs", bufs=6)
                    nc.sync.dma_start(
                        out=xs,
                        in_=xt[jg2 * 256 : (jg2 + 1) * 256, tsl]
                        .rearrange("(jj p) s -> p jj s", p=128)
                        .bitcast(WDT),
                    )
                    if t == 0 and jg2 % 2 == 0:
                        # wqkv stationary tiles as independent halves so the
                        # first matmuls only wait on the half they read
                        jg = jg2 // 2
                        wt[jg] = []
                        for half in range(2):
                            w_ = pp.tile(
                                [128, 2, 768], WDT, tag=f"w{jg}_{half}",
                                name=f"w{jg}_{half}",
                            )
                            nc.sync.dma_start(
                                out=w_,
                                in_=wqkv[
                                    jg * 512 + half * 256 : jg * 512
                                    + half * 256 + 256,
                                    :,
                                ]
                                .rearrange("(jj p) n -> p jj n", p=128)
                                .bitcast(WDT),
                            )
                            wt[jg].append(w_)
                    for jj in range(2):
                        jga, jja = jg2 // 2, (jg2 % 2) * 2 + jj
                        for c in cols:
                            nc.tensor.matmul(
                                psl[c],
                                wt[jga][jja // 2][:, jja % 2, c * 128 : (c + 1) * 128],
                                xs[:, jj, :],
                                start=(jg2 == 0 and jj == 0),
                                stop=(jg2 == 15 and jj == 1),
                            )
                for c in cols:
                    dst = qkv[c][t] if c < 5 else vtmp
                    nc.vector.tensor_copy(dst, psl[c].bitcast(F32R))
            # RoPE on the 4 q heads + k (v stays raw).  Rows 0:64 hold the
            # de-interleaved even (real) lanes, 64:128 the odd (imag) lanes.
            # out = x*[c;c] + swap(x)*[-s;s]; the half-swap crosses
            # partitions so it runs as an SBUF->SBUF DMA.
            for c in (4, 0, 1, 2, 3):
                xsw = pp.tile([128, 512], F32R, tag="xsw", bufs=2)
                nc.sync.dma_start(out=xsw[0:64, :], in_=qkv[c][t][64:128, :])
                nc.sync.dma_start(out=xsw[64:128, :], in_=qkv[c][t][0:64, :])
                m1 = pp.tile([128, 512], F32R, tag="m1", bufs=2)
                m2 = pp.tile([128, 512], F32R, tag="m2", bufs=2)
                nc.vector.tensor_mul(m1, qkv[c][t], cos_sb)
                nc.vector.tensor_mul(m2, xsw, sin_sb)
                nc.vector.tensor_add(qkv[c][t], m1, m2)
            # v in natural [token, hd] layout via PE transposes
            b, t2 = t // 2, t % 2
            for jj in range(4):
                trp = ps.tile([128, 128], F32R, tag="st", bufs=2)
                nc.tensor.transpose(
                    trp, vtmp[:, jj * 128 : (jj + 1) * 128], iden_sb
                )
                v_ = pp.tile([128, 128], BF16, tag="vsb", bufs=16,
                             name=f"vsb{t}_{jj}")
                nc.vector.tensor_copy(v_, trp)
                vsb[(b, t2 * 4 + jj)] = v_

        def attn_unit(t):
            b, t2 = t // 2, t % 2
            jmax = (t2 + 1) * 4
            for h in range(NHC):
                outp = ps.tile([128, 512], F32, tag="ov", bufs=2,
                               name=f"outp{t}_{h}")
                sums = ps.tile([128, 512], F32, tag="ov", bufs=2,
                               name=f"sums{t}_{h}")
                qt = qkv[h][t]
                for j in range(jmax):
                    kt = qkv[4][b * 2 + j // 4]
                    qstart = max(t2 * 512, j * 128)   # in batch tokens
                    width = t2 * 512 + 512 - qstart
                    qoff = qstart - t2 * 512          # within chunk
                    st = ps.tile([128, 512], F32, tag="st", bufs=2)
                    nc.tensor.matmul(
                        st[:, :width],
                        kt[:, (j % 4) * 128 : (j % 4 + 1) * 128],
                        qt[:, qoff : qoff + width],
                        start=True,
                        stop=True,
                    )
                    pexp = pp.tile([128, 512], BF16, tag="pexp", bufs=3)
                    nc.scalar.activation(
                        pexp[:, :width], st[:, :width], EXP, scale=SCALE
                    )
                    if j * 128 >= t2 * 512:
                        # zero the non-causal lower triangle of the
                        # diagonal block (mask_sb is 0/1 here)
                        nc.vector.tensor_mul(
                            pexp[:, 0:128], pexp[:, 0:128], mask_sb
                        )
                    nc.tensor.matmul(
                        outp[:, qoff : qoff + width],
                        vsb[(b, j)],
                        pexp[:, :width],
                        start=(j == 0),
                        stop=(j == jmax - 1),
                    )
                    nc.tensor.matmul(
                        sums[:, qoff : qoff + width],
                        ones_sb,
                        pexp[:, :width],
                        start=(j == 0),
                        stop=(j == jmax - 1),
                    )
                rec = pp.tile([128, 512], F32, tag="rec", bufs=1)
                nc.vector.reciprocal(rec, sums)
                nc.vector.tensor_mul(
                    ao[h][:, t * 512 : (t + 1) * 512], outp, rec
                )

        def ag_chunk(t):
            for h in range(NHC):
                nc.sync.dma_start(
                    out=ag_in[t][h * 128 : (h + 1) * 128, :],
                    in_=ao[h][:, t * 512 : (t + 1) * 512],
                )
            if collective:
                nc.gpsimd.collective_compute(
                    "AllGather",
                    mybir.AluOpType.bypass,
                    ins=[ag_in[t][:, :]],
                    outs=[ag_out[t][:, :]],
                    replica_groups=[list(range(N_CORES))],
                )
            else:
                nc.gpsimd.dma_start(out=ag_out[t][0:512, :], in_=ag_in[t][:, :])

        def wo_unit(b, t2):
            yp = [
                ps.tile([128, 512], F32, tag="acc", bufs=4, name=f"yp{b}{t2}_{d}")
                for d in range(4)
            ]
            for jg in range(8):
                ags = pp.tile([128, 4, 512], WDT, tag="ags", bufs=4)
                nc.gpsimd.dma_start(
                    out=ags,
                    in_=ag_out[b * 2 + t2][
                        jg * 512 : (jg + 1) * 512, :
                    ].rearrange("(jj p) s -> p jj s", p=128),
                )
                for jjj in range(4):
                    for d in range(4):
                        nc.tensor.matmul(
                            yp[d],
                            wt2[jg][:, jjj, d * 128 : (d + 1) * 128],
                            ags[:, jjj, :],
                            start=(jg == 0 and jjj == 0),
                            stop=(jg == 7 and jjj == 3),
                        )
            for d in range(4):
                ys = pp.tile([128, 512], F32, tag="ys", bufs=4)
                if d % 2 == 0:
                    nc.vector.tensor_copy(ys, yp[d])
                else:
                    nc.scalar.copy(ys, yp[d])
                nc.sync.dma_start(
                    out=y[
                        d * 128 : (d + 1) * 128,
                        b * 1024 + t2 * 512 : b * 1024 + (t2 + 1) * 512,
                    ],
                    in_=ys,
                )

        proj_chunk(0)
        if phases >= 2:
            attn_unit(0)
            ag_chunk(0)
        proj_chunk(1)
        if phases >= 2:
            attn_unit(1)
            ag_chunk(1)
        if phases >= 3:
            for jg in range(8):
                nc.gpsimd.dma_start(
                    out=wt2[jg],
                    in_=wo[jg * 512 : (jg + 1) * 512, :]
                    .rearrange("(jj p) n -> p jj n", p=128)
                    .bitcast(WDT),
                )
        proj_chunk(2)
        if phases >= 2:
            attn_unit(2)
            ag_chunk(2)
        proj_chunk(3)
        if phases >= 2:
            attn_unit(3)
            ag_chunk(3)
        if phases >= 3:
            for b in range(B):
                for t2 in range(2):
                    wo_unit(b, t2)


def _build(single=False, phases=3):
    key = ("nc_single" if single else "nc") + str(phases)
    if key in _CACHE:
        return _CACHE[key]
    nc = bacc.Bacc(
        "TRN2",
        target_bir_lowering=False,
        debug=False,
        num_devices=1 if single else N_CORES,
    )
    xt = nc.declare_dram_parameter("xt", [D, TOK], WDT, isOutput=False)
    wqkv = nc.declare_dram_parameter("wqkv", [D, 768], WDT, isOutput=False)
    wo = nc.declare_dram_parameter("wo", [D, 512], WDT, isOutput=False)
    cost = nc.declare_dram_parameter("cost", [128, TOK], F32, isOutput=False)
    sint = nc.declare_dram_parameter("sint", [128, TOK], F32, isOutput=False)
    mask = nc.declare_dram_parameter("mask", [128, 128], BF16, isOutput=False)
    iden = nc.declare_dram_parameter("iden", [128, 128], F32, isOutput=False)
    ones = nc.declare_dram_parameter("ones", [128, 128], BF16, isOutput=False)
    y = nc.declare_dram_parameter("y", [512, TOK], F32, isOutput=True)
    with tile.TileContext(nc) as tc:
        _emit(
            nc,
            tc,
            (xt, wqkv, wo, cost, sint, mask, iden, ones, y),
            collective=not single,
            phases=phases,
        )
    nc.compile()
    _CACHE[key] = nc
    return nc


def _host_inputs(x, wq, wk, wv, wo, freqs_cos, freqs_sin):
    """Build the per-core input maps (host-side sharding/layout prep)."""
    import ml_dtypes

    wnp = ml_dtypes.bfloat16 if USE_BF16 else np.float32
    xt = np.ascontiguousarray(x.reshape(TOK, D).T).astype(wnp)  # [D, TOK]
    # de-interleave permutation of the head dim for q/k weight columns
    perm = np.concatenate([np.arange(0, HD, 2), np.arange(1, HD, 2)])
    cos_t = np.tile(freqs_cos.T, (1, B))  # [64, TOK]
    sin_t = np.tile(freqs_sin.T, (1, B))
    cost = np.concatenate([cos_t, cos_t], axis=0).astype(np.float32)  # [128, TOK]
    sint = np.concatenate([-sin_t, sin_t], axis=0).astype(np.float32)
    kq, qq = np.meshgrid(np.arange(128), np.arange(128), indexing="ij")
    mask = np.where(qq >= kq, 1.0, 0.0).astype(wnp)  # [k, q], 0/1
    iden = np.eye(128, dtype=np.float32)
    ones = np.ones((128, 128), wnp)

    in_maps = []
    for c in range(N_CORES):
        wq_c = wq[:, c * 512 : (c + 1) * 512].reshape(D, NHC, HD)[:, :, perm]
        wq_c = wq_c.reshape(D, NHC * HD)
        wk_c = wk[:, c * 128 : (c + 1) * 128][:, perm]
        wv_c = wv[:, c * 128 : (c + 1) * 128]
        wqkv_c = np.ascontiguousarray(
            np.concatenate([wq_c, wk_c, wv_c], axis=1)
        ).astype(wnp)  # [D, 768]
        wo_c = np.ascontiguousarray(wo[:, c * 512 : (c + 1) * 512]).astype(wnp)
        in_maps.append(
            {
                "xt": xt,
                "wqkv": wqkv_c,
                "wo": wo_c,
                "cost": cost,
                "sint": sint,
                "mask": mask,
                "iden": iden,
                "ones": ones,
            }
        )
    return in_maps


def kernel(
    x,
    wq,
    wk,
    wv,
    wo,
    freqs_cos,
    freqs_sin,
    cache_k=None,
    cache_v=None,
    start_pos=0,
):
    # start_pos is 0 in this problem; the cache read-back region is then
    # exactly the freshly written k/v, so the caches never matter.
    assert int(start_pos) == 0
    x = np.asarray(x, np.float32)
    in_maps = _host_inputs(
        x,
        np.asarray(wq, np.float32),
        np.asarray(wk, np.float32),
        np.asarray(wv, np.float32),
        np.asarray(wo, np.float32),
        np.asarray(freqs_cos, np.float32),
        np.asarray(freqs_sin, np.float32),
    )
    nc = _build()
    res = run_bass_kernel_spmd(nc, in_maps, list(range(N_CORES))).results
    y_t = np.concatenate([res[c]["y"] for c in range(N_CORES)], axis=0)  # [D, TOK]
    return np.ascontiguousarray(y_t.T).reshape(B, S, D).astype(np.float32)


# revision 18
# speedup vs baseline: 1.9657x; 1.0598x over previous
"""Trainium2 Bass kernel for llama-style GQA attention layer (B=2, S=1024,
D=4096, H=32, KVH=8, HD=128, start_pos=0), tensor-parallel over heads on 8
NeuronCores.

Per-core plan (core c of 8):
  - owns q heads 4c..4c+3 (wq cols 512c..512c+512) and kv head c
    (wk/wv cols 128c..128c+128); GQA grouping means exactly its heads use
    exactly its kv head.
  - computes qT/kT/vT = (x @ w).T directly in [col, token] layout via
    w-stationary matmuls against host-transposed xT.
  - RoPE applied in qT/kT layout; the head-dim interleave is de-interleaved
    by permuting wq/wk columns on the host (a consistent permutation of the
    hd axis of q AND k leaves q.k dot products unchanged).  The half-swap
    runs as an SBUF->SBUF partition-crossing DMA (not on the PE).
  - attention computed in transposed orientation: ST[k, q] from kT-stationary
    matmuls, softmax without max-subtraction (scores are O(10) here so fp32
    exp is safe), PV with v-stationary matmuls giving outT [hd, q]; softmax
    denominators come from an all-ones [128,128]-stationary matmul whose
    output IS the row-sum broadcast across partitions (no separate
    broadcast step), then reciprocal+multiply on the DVE.
  - heads' outT concat -> AllGather (per batch) -> full attn_outT on every
    core -> each core computes a 512-column slice of the wo projection
    (no all-reduce needed: the head contraction happens inside the matmul).
Host unshards by concatenating the 8 column slices and transposing.

The whole kernel is emitted as ONE interleaved stream: for each 512-token
chunk, projection -> RoPE -> that chunk's attention unit, with the wo
projection units at the end.  The Tile scheduler then fills attention's
exp/normalization dependency stalls with projection / wo matmuls, keeping
the PE (the bottleneck engine: ~320 us of matmul work per core) dense
instead of serializing three phases.  PSUM is split 4/2/2 banks:
projection runs its 6 output columns in two groups of 3 (streaming xT from
DRAM twice) sharing a 4-slot accumulator tag with the wo units, scores get
2, and out/sums accumulators 2.

Projection and wo matmuls run in bf16 (fp32 PSUM accumulation); the score
path (q/k, exp input) stays fp32r/fp32, and the post-softmax path (probs,
v) is bf16.
"""

import numpy as np
import concourse.mybir as mybir
import concourse.tile as tile
from concourse import bacc
from concourse.bass_utils import run_bass_kernel_spmd

N_CORES = 8
B, S, D = 2, 1024, 4096
TOK = B * S            # 2048 flattened tokens
HD = 128
NHC = 4                # q heads per core
CH = 4                 # 512-token chunks
F32 = mybir.dt.float32
F32R = mybir.dt.float32r
BF16 = mybir.dt.bfloat16
USE_BF16 = True          # bf16 projection/wo matmuls (fp32 PSUM accumulate)
WDT = BF16 if USE_BF16 else F32R
SCALE = 1.0 / float(np.sqrt(128.0))

_CACHE = {}


def _emit(nc, tc, aps, collective=True, phases=3, sfx=""):
    xt, wqkv, wo, cost, sint, mask, iden, ones, y = aps
    # one AllGather per 512-token chunk: fires as soon as that chunk's
    # attention completes, so every collective has a long compute window
    # (the following chunks' projection/attention + earlier wo units) to
    # hide under.
    ag_in = [nc.dram_tensor(f"ag_in{t}{sfx}", [512, 512], WDT) for t in range(CH)]
    ag_out = [
        nc.dram_tensor(f"ag_out{t}{sfx}", [D, 512], WDT, addr_space="Shared")
        for t in range(CH)
    ]
    EXP = mybir.ActivationFunctionType.Exp

    with tc.tile_pool(name="pp" + sfx, bufs=1) as pp, tc.tile_pool(
        name="ps" + sfx, bufs=1, space="PSUM"
    ) as ps:
        mask_sb = pp.tile([128, 128], BF16, tag="mask")
        iden_sb = pp.tile([128, 128], F32R, tag="iden")
        ones_sb = pp.tile([128, 128], BF16, tag="ones")
        nc.gpsimd.dma_start(out=mask_sb, in_=mask[:, :])
        nc.gpsimd.dma_start(out=iden_sb, in_=iden[:, :].bitcast(F32R))
        nc.gpsimd.dma_start(out=ones_sb, in_=ones[:, :])

        # persistent SBUF state
        qkv = [
            [pp.tile([128, 512], F32R, tag=f"qkv{c}_{t}", name=f"qkv{c}_{t}") for t in range(CH)]
            for c in range(5)
        ]  # 4 q heads + k, per chunk, RoPE'd in place
        vsb = {}  # (b, j) -> [tok128, hd128] bf16 v blocks
        ao = [pp.tile([128, TOK], WDT, tag=f"ao{h}", name=f"ao{h}") for h in range(NHC)]
        wt = [None] * 8
        wt2 = [pp.tile([128, 4, 512], WDT, tag=f"wo{jg}", name=f"wo{jg}") for jg in range(8)]

        def proj_chunk(t):
            tsl = slice(t * 512, (t + 1) * 512)
            cos_sb = pp.tile([128, 512], F32R, tag="cos", bufs=2)
            sin_sb = pp.tile([128, 512], F32R, tag="sin", bufs=2)
            nc.scalar.dma_start(out=cos_sb, in_=cost[:, tsl].bitcast(F32R))
            nc.scalar.dma_start(out=sin_sb, in_=sint[:, tsl].bitcast(F32R))
            vtmp = pp.tile([128, 512], F32R, tag="vtmp", bufs=2, name=f"vtmp{t}")
            # Chunk 0 runs all 6 output columns in one pass over xT
            # (borrowing 2 idle score banks — attention hasn't started yet)
            # to halve the DMA-bound startup window.  Later chunks use two
            # col-groups of 3 so only 4 accumulator banks are needed while
            # attention/wo own the rest; xT streams once per group.
            groups = [(0, 1, 2, 3, 4, 5)] if t == 0 else [(0, 1, 2), (3, 4, 5)]
            for cols in groups:
                psl = {
                    c: ps.tile(
                        [128, 512], F32,
                        tag=("acc" if c < 4 or t > 0 else "st"),
                        bufs=(4 if c < 4 or t > 0 else 2),
                        name=f"psl{t}_{c}",
                    )
                    for c in cols
                }
                for jg2 in range(16):
                    xs = pp.tile([128, 2, 512], WDT, tag="xs", bufs=6)
                    nc.sync.dma_start(
                        out=xs,
                        in_=xt[jg2 * 256 : (jg2 + 1) * 256, tsl]
                        .rearrange("(jj p) s -> p jj s", p=128)
                        .bitcast(WDT),
                    )
                    if t == 0 and jg2 % 2 == 0:
                        # wqkv stationary tiles as independent halves so the
                        # first matmuls only wait on the half they read
                        jg = jg2 // 2
                        wt[jg] = []
                        for half in range(2):
                            w_ = pp.tile(
                                [128, 2, 768], WDT, tag=f"w{jg}_{half}",
                                name=f"w{jg}_{half}",
                            )
                            nc.sync.dma_start(
                                out=w_,
                                in_=wqkv[
                                    jg * 512 + half * 256 : jg * 512
                                    + half * 256 + 256,
                                    :,
                                ]
                                .rearrange("(jj p) n -> p jj n", p=128)
                                .bitcast(WDT),
                            )
                            wt[jg].append(w_)
                    for jj in range(2):
                        jga, jja = jg2 // 2, (jg2 % 2) * 2 + jj
                        for c in cols:
                            nc.tensor.matmul(
                                psl[c],
                                wt[jga][jja // 2][:, jja % 2, c * 128 : (c + 1) * 128],
                                xs[:, jj, :],
                                start=(jg2 == 0 and jj == 0),
                                stop=(jg2 == 15 and jj == 1),
                            )
                for c in cols:
                    dst = qkv[c][t] if c < 5 else vtmp
                    nc.vector.tensor_copy(dst, psl[c].bitcast(F32R))
            # RoPE on the 4 q heads + k (v stays raw).  Rows 0:64 hold the
            # de-interleaved even (real) lanes, 64:128 the odd (imag) lanes.
            # out = x*[c;c] + swap(x)*[-s;s]; the half-swap crosses
            # partitions so it runs as an SBUF->SBUF DMA.
            for c in (4, 0, 1, 2, 3):
                xsw = pp.tile([128, 512], F32R, tag="xsw", bufs=2)
                nc.sync.dma_start(out=xsw[0:64, :], in_=qkv[c][t][64:128, :])
                nc.sync.dma_start(out=xsw[64:128, :], in_=qkv[c][t][0:64, :])
                m1 = pp.tile([128, 512], F32R, tag="m1", bufs=2)
                m2 = pp.tile([128, 512], F32R, tag="m2", bufs=2)
                nc.vector.tensor_mul(m1, qkv[c][t], cos_sb)
                nc.vector.tensor_mul(m2, xsw, sin_sb)
                nc.vector.tensor_add(qkv[c][t], m1, m2)
            # v in natural [token, hd] layout via PE transposes
            b, t2 = t // 2, t % 2
            for jj in range(4):
                trp = ps.tile([128, 128], F32R, tag="st", bufs=2)
                nc.tensor.transpose(
                    trp, vtmp[:, jj * 128 : (jj + 1) * 128], iden_sb
                )
                v_ = pp.tile([128, 128], BF16, tag="vsb", bufs=16,
                             name=f"vsb{t}_{jj}")
                nc.vector.tensor_copy(v_, trp)
                vsb[(b, t2 * 4 + jj)] = v_

        def attn_unit(t):
            b, t2 = t // 2, t % 2
            jmax = (t2 + 1) * 4
            for h in range(NHC):
                outp = ps.tile([128, 512], F32, tag="ov", bufs=2,
                               name=f"outp{t}_{h}")
                sums = ps.tile([128, 512], F32, tag="ov", bufs=2,
                               name=f"sums{t}_{h}")
                qt = qkv[h][t]
                for j in range(jmax):
                    kt = qkv[4][b * 2 + j // 4]
                    qstart = max(t2 * 512, j * 128)   # in batch tokens
                    width = t2 * 512 + 512 - qstart
                    qoff = qstart - t2 * 512          # within chunk
                    st = ps.tile([128, 512], F32, tag="st", bufs=2)
                    nc.tensor.matmul(
                        st[:, :width],
                        kt[:, (j % 4) * 128 : (j % 4 + 1) * 128],
                        qt[:, qoff : qoff + width],
                        start=True,
                        stop=True,
                    )
                    pexp = pp.tile([128, 512], BF16, tag="pexp", bufs=3)
                    nc.scalar.activation(
                        pexp[:, :width], st[:, :width], EXP, scale=SCALE
                    )
                    if j * 128 >= t2 * 512:
                        # zero the non-causal lower triangle of the
                        # diagonal block (mask_sb is 0/1 here)
                        nc.vector.tensor_mul(
                            pexp[:, 0:128], pexp[:, 0:128], mask_sb
                        )
                    nc.tensor.matmul(
                        outp[:, qoff : qoff + width],
                        vsb[(b, j)],
                        pexp[:, :width],
                        start=(j == 0),
                        stop=(j == jmax - 1),
                    )
                    nc.tensor.matmul(
                        sums[:, qoff : qoff + width],
                        ones_sb,
                        pexp[:, :width],
                        start=(j == 0),
                        stop=(j == jmax - 1),
                    )
                rec = pp.tile([128, 512], F32, tag="rec", bufs=1)
                nc.vector.reciprocal(rec, sums)
                nc.vector.tensor_mul(
                    ao[h][:, t * 512 : (t + 1) * 512], outp, rec
                )

        def ag_chunk(t):
            for h in range(NHC):
                nc.sync.dma_start(
                    out=ag_in[t][h * 128 : (h + 1) * 128, :],
                    in_=ao[h][:, t * 512 : (t + 1) * 512],
                )
            if collective:
                nc.gpsimd.collective_compute(
                    "AllGather",
                    mybir.AluOpType.bypass,
                    ins=[ag_in[t][:, :]],
                    outs=[ag_out[t][:, :]],
                    replica_groups=[list(range(N_CORES))],
                )
            else:
                nc.gpsimd.dma_start(out=ag_out[t][0:512, :], in_=ag_in[t][:, :])

        def wo_unit(b, t2):
            yp = [
                ps.tile([128, 512], F32, tag="acc", bufs=4, name=f"yp{b}{t2}_{d}")
                for d in range(4)
            ]
            for jg in range(8):
                ags = pp.tile([128, 4, 512], WDT, tag="ags", bufs=4)
                nc.scalar.dma_start(
                    out=ags,
                    in_=ag_out[b * 2 + t2][
                        jg * 512 : (jg + 1) * 512, :
                    ].rearrange("(jj p) s -> p jj s", p=128),
                )
                for jjj in range(4):
                    for d in range(4):
                        nc.tensor.matmul(
                            yp[d],
                            wt2[jg][:, jjj, d * 128 : (d + 1) * 128],
                            ags[:, jjj, :],
                            start=(jg == 0 and jjj == 0),
                            stop=(jg == 7 and jjj == 3),
                        )
            for d in range(4):
                ys = pp.tile([128, 512], F32, tag="ys", bufs=4)
                if d % 2 == 0:
                    nc.vector.tensor_copy(ys, yp[d])
                else:
                    nc.scalar.copy(ys, yp[d])
                nc.sync.dma_start(
                    out=y[
                        d * 128 : (d + 1) * 128,
                        b * 1024 + t2 * 512 : b * 1024 + (t2 + 1) * 512,
                    ],
                    in_=ys,
                )

        proj_chunk(0)
        if phases >= 2:
            attn_unit(0)
            ag_chunk(0)
        proj_chunk(1)
        if phases >= 2:
            attn_unit(1)
            ag_chunk(1)
        if phases >= 3:
            for jg in range(8):
                nc.scalar.dma_start(
                    out=wt2[jg],
                    in_=wo[jg * 512 : (jg + 1) * 512, :]
                    .rearrange("(jj p) n -> p jj n", p=128)
                    .bitcast(WDT),
                )
        proj_chunk(2)
        if phases >= 2:
            attn_unit(2)
            ag_chunk(2)
        proj_chunk(3)
        if phases >= 2:
            attn_unit(3)
            ag_chunk(3)
        if phases >= 3:
            for b in range(B):
                for t2 in range(2):
                    wo_unit(b, t2)


def _build(single=False, phases=3):
    key = ("nc_single" if single else "nc") + str(phases)
    if key in _CACHE:
        return _CACHE[key]
    nc = bacc.Bacc(
        "TRN2",
        target_bir_lowering=False,
        debug=False,
        num_devices=1 if single else N_CORES,
    )
    xt = nc.declare_dram_parameter("xt", [D, TOK], WDT, isOutput=False)
    wqkv = nc.declare_dram_parameter("wqkv", [D, 768], WDT, isOutput=False)
    wo = nc.declare_dram_parameter("wo", [D, 512], WDT, isOutput=False)
    cost = nc.declare_dram_parameter("cost", [128, TOK], F32, isOutput=False)
    sint = nc.declare_dram_parameter("sint", [128, TOK], F32, isOutput=False)
    mask = nc.declare_dram_parameter("mask", [128, 128], BF16, isOutput=False)
    iden = nc.declare_dram_parameter("iden", [128, 128], F32, isOutput=False)
    ones = nc.declare_dram_parameter("ones", [128, 128], BF16, isOutput=False)
    y = nc.declare_dram_parameter("y", [512, TOK], F32, isOutput=True)
    with tile.TileContext(nc) as tc:
        _emit(
            nc,
            tc,
            (xt, wqkv, wo, cost, sint, mask, iden, ones, y),
            collective=not single,
            phases=phases,
        )
    nc.compile()
    _CACHE[key] = nc
    return nc


def _host_inputs(x, wq, wk, wv, wo, freqs_cos, freqs_sin):
    """Build the per-core input maps (host-side sharding/layout prep)."""
    import ml_dtypes

    wnp = ml_dtypes.bfloat16 if USE_BF16 else np.float32
    xt = np.ascontiguousarray(x.reshape(TOK, D).T).astype(wnp)  # [D, TOK]
    # de-interleave permutation of the head dim for q/k weight columns
    perm = np.concatenate([np.arange(0, HD, 2), np.arange(1, HD, 2)])
    cos_t = np.tile(freqs_cos.T, (1, B))  # [64, TOK]
    sin_t = np.tile(freqs_sin.T, (1, B))
    cost = np.concatenate([cos_t, cos_t], axis=0).astype(np.float32)  # [128, TOK]
    sint = np.concatenate([-sin_t, sin_t], axis=0).astype(np.float32)
    kq, qq = np.meshgrid(np.arange(128), np.arange(128), indexing="ij")
    mask = np.where(qq >= kq, 1.0, 0.0).astype(wnp)  # [k, q], 0/1
    iden = np.eye(128, dtype=np.float32)
    ones = np.ones((128, 128), wnp)

    in_maps = []
    for c in range(N_CORES):
        wq_c = wq[:, c * 512 : (c + 1) * 512].reshape(D, NHC, HD)[:, :, perm]
        wq_c = wq_c.reshape(D, NHC * HD)
        wk_c = wk[:, c * 128 : (c + 1) * 128][:, perm]
        wv_c = wv[:, c * 128 : (c + 1) * 128]
        wqkv_c = np.ascontiguousarray(
            np.concatenate([wq_c, wk_c, wv_c], axis=1)
        ).astype(wnp)  # [D, 768]
        wo_c = np.ascontiguousarray(wo[:, c * 512 : (c + 1) * 512]).astype(wnp)
        in_maps.append(
            {
                "xt": xt,
                "wqkv": wqkv_c,
                "wo": wo_c,
                "cost": cost,
                "sint": sint,
                "mask": mask,
                "iden": iden,
                "ones": ones,
            }
        )
    return in_maps


def kernel(
    x,
    wq,
    wk,
    wv,
    wo,
    freqs_cos,
    freqs_sin,
    cache_k=None,
    cache_v=None,
    start_pos=0,
):
    # start_pos is 0 in this problem; the cache read-back region is then
    # exactly the freshly written k/v, so the caches never matter.
    assert int(start_pos) == 0
    x = np.asarray(x, np.float32)
    in_maps = _host_inputs(
        x,
        np.asarray(wq, np.float32),
        np.asarray(wk, np.float32),
        np.asarray(wv, np.float32),
        np.asarray(wo, np.float32),
        np.asarray(freqs_cos, np.float32),
        np.asarray(freqs_sin, np.float32),
    )
    nc = _build()
    res = run_bass_kernel_spmd(nc, in_maps, list(range(N_CORES))).results
    y_t = np.concatenate([res[c]["y"] for c in range(N_CORES)], axis=0)  # [D, TOK]
    return np.ascontiguousarray(y_t.T).reshape(B, S, D).astype(np.float32)
